# revision 5
# baseline (speedup 1.0000x reference)
"""Trainium2 Bass kernel for nn_ExampleTiedDropout (gather rows + multiply).

out[b] = X[b] * mask_tensor[idx[b]]   (elementwise, f32)

Strategy: data-parallel over batch. 8 cores, 512 examples each; the mask
table is replicated to every core's HBM.

Two device kernels:
 - "fused": per 128-example tile, DMA X tile [128, 2048] to SBUF, then an
   indirect-DMA gather of full 8KB mask rows with compute_op=mult (CCE
   multiplies the gathered stream onto the X tile during the DMA), then
   store. Works for arbitrary mask tables.
 - "compact": the reference's mask rows are constant across H*W within a
   channel (bernoulli value broadcast), so only C=32 floats per row are
   distinct. Host slices mask[:, :, 0, 0] into a [60000, 32] compact
   table (verified exactly against the full table; falls back to "fused"
   if the structure doesn't hold), the device gathers 128B/example and
   expands via a broadcast access pattern on VectorE. 3x less gather
   traffic.
"""

import os

import numpy as np

import concourse.bacc as bacc
import concourse.bass as bass
import concourse.mybir as mybir
import concourse.tile as tile
from concourse.bass_utils import run_bass_kernel_spmd

B, C, H, W = 4096, 32, 8, 8
MAX_ID = 60000
HW = H * W  # 64
D = C * HW  # 2048 f32 = 8KB per row
N_CORES = 8
BS = B // N_CORES  # 512 examples per core
P = 128
NBLK = BS // P  # 4 tiles of 128 examples

_cache = {}


def _build_fused(use_cce_mult=True):
    nc = bacc.Bacc(None, target_bir_lowering=False)
    x_d = nc.dram_tensor("x", [BS, D], mybir.dt.float32, kind="ExternalInput")
    idx_d = nc.dram_tensor("idx", [P, NBLK], mybir.dt.int32, kind="ExternalInput")
    mask_d = nc.dram_tensor(
        "mask", [MAX_ID, D], mybir.dt.float32, kind="ExternalInput"
    )
    out_d = nc.dram_tensor("out", [BS, D], mybir.dt.float32, kind="ExternalOutput")

    with tile.TileContext(nc) as tc:
        with (
            tc.tile_pool(name="idxp", bufs=1) as idxp,
            tc.tile_pool(name="sbuf", bufs=NBLK) as pool,
        ):
            idx_t = idxp.tile([P, NBLK], mybir.dt.int32)
            nc.sync.dma_start(out=idx_t[:], in_=idx_d[:])

            for b in range(NBLK):
                sl = slice(b * P, (b + 1) * P)
                x_t = pool.tile([P, D], mybir.dt.float32, tag="x")
                nc.sync.dma_start(out=x_t[:], in_=x_d[sl, :])
                if use_cce_mult:
                    # gather mask rows and multiply onto x_t in the DMA
                    nc.gpsimd.indirect_dma_start(
                        out=x_t[:],
                        out_offset=None,
                        in_=mask_d[:],
                        in_offset=bass.IndirectOffsetOnAxis(
                            ap=idx_t[:, b : b + 1], axis=0
                        ),
                        compute_op=mybir.AluOpType.mult,
                    )
                    nc.scalar.dma_start(out=out_d[sl, :], in_=x_t[:])
                else:
                    m_t = pool.tile([P, D], mybir.dt.float32, tag="m")
                    nc.gpsimd.indirect_dma_start(
                        out=m_t[:],
                        out_offset=None,
                        in_=mask_d[:],
                        in_offset=bass.IndirectOffsetOnAxis(
                            ap=idx_t[:, b : b + 1], axis=0
                        ),
                    )
                    o_t = pool.tile([P, D], mybir.dt.float32, tag="o")
                    nc.vector.tensor_mul(out=o_t[:], in0=x_t[:], in1=m_t[:])
                    nc.scalar.dma_start(out=out_d[sl, :], in_=o_t[:])
    nc.finalize()
    return nc


def _build_compact(split=2):
    """split: free-dim chunks per 128-example block (channels split
    C//split at a time) for finer load->mult->store pipelining."""
    nc = bacc.Bacc(None, target_bir_lowering=False)
    x_d = nc.dram_tensor("x", [BS, D], mybir.dt.float32, kind="ExternalInput")
    idx_d = nc.dram_tensor("idx", [P, NBLK], mybir.dt.int32, kind="ExternalInput")
    mask_d = nc.dram_tensor(
        "mask", [MAX_ID, C], mybir.dt.float32, kind="ExternalInput"
    )
    out_d = nc.dram_tensor("out", [BS, D], mybir.dt.float32, kind="ExternalOutput")

    CS = C // split  # channels per chunk
    DS = D // split  # elements per chunk

    with tile.TileContext(nc) as tc:
        with (
            tc.tile_pool(name="idxp", bufs=1) as idxp,
            tc.tile_pool(name="mp", bufs=NBLK) as mp,
            tc.tile_pool(name="sbuf", bufs=NBLK) as pool,
        ):
            idx_t = idxp.tile([P, NBLK], mybir.dt.int32)
            nc.sync.dma_start(out=idx_t[:], in_=idx_d[:])

            for b in range(NBLK):
                sl = slice(b * P, (b + 1) * P)
                x_t = pool.tile([P, D], mybir.dt.float32, tag="x")
                m_t = mp.tile([P, C], mybir.dt.float32, tag="m")
                nc.gpsimd.indirect_dma_start(
                    out=m_t[:],
                    out_offset=None,
                    in_=mask_d[:],
                    in_offset=bass.IndirectOffsetOnAxis(
                        ap=idx_t[:, b : b + 1], axis=0
                    ),
                )
                x_3d = x_t[:].rearrange("p (c j) -> p c j", c=C)
                for s in range(split):
                    cs = slice(s * DS, (s + 1) * DS)
                    nc.sync.dma_start(out=x_t[:, cs], in_=x_d[sl, cs])
                    # in1[p, c, j] = m_t[p, c]  (step-0 inner axis)
                    m_bc = m_t[:, s * CS : (s + 1) * CS, None].to_broadcast(
                        [P, CS, HW]
                    )
                    # in-place multiply into the X tile
                    nc.vector.tensor_mul(
                        out=x_3d[:, s * CS : (s + 1) * CS, :],
                        in0=x_3d[:, s * CS : (s + 1) * CS, :],
                        in1=m_bc,
                    )
                    nc.scalar.dma_start(out=out_d[sl, cs], in_=x_t[:, cs])
    nc.finalize()
    return nc


def _get_nc(variant):
    key = f"nc_{variant}"
    if key not in _cache:
        if variant == "fused":
            _cache[key] = _build_fused(use_cce_mult=True)
        elif variant == "dve":
            _cache[key] = _build_fused(use_cce_mult=False)
        elif variant == "compact":
            _cache[key] = _build_compact()
        else:
            raise ValueError(variant)
    return _cache[key]


def _mask_is_broadcast(mask2):
    # mask rows constant across HW within each channel?
    m4 = mask2.reshape(MAX_ID, C, HW)
    # sample check first to fail fast, then full check
    s = m4[::997]
    if not np.all(s == s[:, :, :1]):
        return False
    return bool(np.all(m4 == m4[:, :, :1]))


def kernel(X, idx, mask_tensor, _profile=False, _variant=None):
    assert X.shape == (B, C, H, W) and mask_tensor.shape == (MAX_ID, C, H, W)
    X2 = np.ascontiguousarray(np.asarray(X, dtype=np.float32).reshape(B, D))
    mask2 = np.asarray(mask_tensor, dtype=np.float32).reshape(MAX_ID, D)
    idx32 = np.asarray(idx).astype(np.int32).reshape(B)

    variant = _variant or os.environ.get("BASS_VARIANT")
    if variant is None:
        variant = "compact" if _mask_is_broadcast(mask2) else "fused"
    if variant == "compact":
        mask_in = np.ascontiguousarray(mask2[:, ::HW])  # [MAX_ID, C]
    else:
        mask_in = np.ascontiguousarray(mask2)

    nc = _get_nc(variant)

    in_maps = []
    for c in range(N_CORES):
        shard = slice(c * BS, (c + 1) * BS)
        idx_shard = np.ascontiguousarray(idx32[shard].reshape(NBLK, P).T)
        in_maps.append({"x": X2[shard], "idx": idx_shard, "mask": mask_in})

    res = run_bass_kernel_spmd(
        nc, in_maps, core_ids=list(range(N_CORES)), trace=_profile
    )
    out = np.concatenate([r["out"] for r in res.results], axis=0)
    if _profile:
        kernel.last_exec_time_ns = res.exec_time_ns
        kernel.last_results = res
    return out.reshape(B, C, H, W)


# revision 6
# speedup vs baseline: 1.0320x; 1.0320x over previous
"""Trainium2 Bass kernel for nn_ExampleTiedDropout (gather rows + multiply).

out[b] = X[b] * mask_tensor[idx[b]]   (elementwise, f32)

Strategy: data-parallel over batch. 8 cores, 512 examples each; the mask
table is replicated to every core's HBM.

Two device kernels:
 - "fused": per 128-example tile, DMA X tile [128, 2048] to SBUF, then an
   indirect-DMA gather of full 8KB mask rows with compute_op=mult (CCE
   multiplies the gathered stream onto the X tile during the DMA), then
   store. Works for arbitrary mask tables.
 - "compact": the reference's mask rows are constant across H*W within a
   channel (bernoulli value broadcast), so only C=32 floats per row are
   distinct. Host slices mask[:, :, 0, 0] into a [60000, 32] compact
   table (verified exactly against the full table; falls back to "fused"
   if the structure doesn't hold), the device gathers 128B/example and
   expands via a broadcast access pattern on VectorE. 3x less gather
   traffic.
"""

import os

import numpy as np

import concourse.bacc as bacc
import concourse.bass as bass
import concourse.mybir as mybir
import concourse.tile as tile
from concourse.bass_utils import run_bass_kernel_spmd

B, C, H, W = 4096, 32, 8, 8
MAX_ID = 60000
HW = H * W  # 64
D = C * HW  # 2048 f32 = 8KB per row
N_CORES = 8
BS = B // N_CORES  # 512 examples per core
P = 128
NBLK = BS // P  # 4 tiles of 128 examples

_cache = {}


def _build_fused(use_cce_mult=True):
    nc = bacc.Bacc(None, target_bir_lowering=False)
    x_d = nc.dram_tensor("x", [BS, D], mybir.dt.float32, kind="ExternalInput")
    idx_d = nc.dram_tensor("idx", [P, NBLK], mybir.dt.int32, kind="ExternalInput")
    mask_d = nc.dram_tensor(
        "mask", [MAX_ID, D], mybir.dt.float32, kind="ExternalInput"
    )
    out_d = nc.dram_tensor("out", [BS, D], mybir.dt.float32, kind="ExternalOutput")

    with tile.TileContext(nc) as tc:
        with (
            tc.tile_pool(name="idxp", bufs=1) as idxp,
            tc.tile_pool(name="sbuf", bufs=NBLK) as pool,
        ):
            idx_t = idxp.tile([P, NBLK], mybir.dt.int32)
            nc.sync.dma_start(out=idx_t[:], in_=idx_d[:])

            for b in range(NBLK):
                sl = slice(b * P, (b + 1) * P)
                x_t = pool.tile([P, D], mybir.dt.float32, tag="x")
                nc.sync.dma_start(out=x_t[:], in_=x_d[sl, :])
                if use_cce_mult:
                    # gather mask rows and multiply onto x_t in the DMA
                    nc.gpsimd.indirect_dma_start(
                        out=x_t[:],
                        out_offset=None,
                        in_=mask_d[:],
                        in_offset=bass.IndirectOffsetOnAxis(
                            ap=idx_t[:, b : b + 1], axis=0
                        ),
                        compute_op=mybir.AluOpType.mult,
                    )
                    nc.scalar.dma_start(out=out_d[sl, :], in_=x_t[:])
                else:
                    m_t = pool.tile([P, D], mybir.dt.float32, tag="m")
                    nc.gpsimd.indirect_dma_start(
                        out=m_t[:],
                        out_offset=None,
                        in_=mask_d[:],
                        in_offset=bass.IndirectOffsetOnAxis(
                            ap=idx_t[:, b : b + 1], axis=0
                        ),
                    )
                    o_t = pool.tile([P, D], mybir.dt.float32, tag="o")
                    nc.vector.tensor_mul(out=o_t[:], in0=x_t[:], in1=m_t[:])
                    nc.scalar.dma_start(out=out_d[sl, :], in_=o_t[:])
    nc.finalize()
    return nc


def _build_compact(split=2):
    """split: free-dim chunks per 128-example block (channels split
    C//split at a time) for finer load->mult->store pipelining."""
    nc = bacc.Bacc(None, target_bir_lowering=False)
    x_d = nc.dram_tensor("x", [BS, D], mybir.dt.float32, kind="ExternalInput")
    idx_d = nc.dram_tensor("idx", [P, NBLK], mybir.dt.int32, kind="ExternalInput")
    mask_d = nc.dram_tensor(
        "mask", [MAX_ID, C], mybir.dt.float32, kind="ExternalInput"
    )
    out_d = nc.dram_tensor("out", [BS, D], mybir.dt.float32, kind="ExternalOutput")

    CS = C // split  # channels per chunk
    DS = D // split  # elements per chunk

    with tile.TileContext(nc) as tc:
        with (
            tc.tile_pool(name="idxp", bufs=1) as idxp,
            tc.tile_pool(name="mp", bufs=NBLK) as mp,
            tc.tile_pool(name="sbuf", bufs=NBLK * split) as pool,
        ):
            # idx via gpsimd (SWDGE) so the gathers on the same engine can
            # start as early as possible
            idx_t = idxp.tile([P, NBLK], mybir.dt.int32)
            nc.gpsimd.dma_start(out=idx_t[:], in_=idx_d[:])

            for b in range(NBLK):
                sl = slice(b * P, (b + 1) * P)
                m_t = mp.tile([P, C], mybir.dt.float32, tag="m")
                nc.gpsimd.indirect_dma_start(
                    out=m_t[:],
                    out_offset=None,
                    in_=mask_d[:],
                    in_offset=bass.IndirectOffsetOnAxis(
                        ap=idx_t[:, b : b + 1], axis=0
                    ),
                )
                for s in range(split):
                    cs = slice(s * DS, (s + 1) * DS)
                    # per-chunk tile: no false WAR deps between chunks
                    x_t = pool.tile([P, DS], mybir.dt.float32, tag="x")
                    nc.sync.dma_start(out=x_t[:], in_=x_d[sl, cs])
                    # in1[p, c, j] = m_t[p, c]  (step-0 inner axis)
                    m_bc = m_t[:, s * CS : (s + 1) * CS, None].to_broadcast(
                        [P, CS, HW]
                    )
                    x_3d = x_t[:].rearrange("p (c j) -> p c j", c=CS)
                    # in-place multiply into the X chunk tile
                    nc.vector.tensor_mul(out=x_3d, in0=x_3d, in1=m_bc)
                    nc.scalar.dma_start(out=out_d[sl, cs], in_=x_t[:])
    nc.finalize()
    return nc


def _get_nc(variant):
    key = f"nc_{variant}"
    if key not in _cache:
        if variant == "fused":
            _cache[key] = _build_fused(use_cce_mult=True)
        elif variant == "dve":
            _cache[key] = _build_fused(use_cce_mult=False)
        elif variant == "compact":
            _cache[key] = _build_compact()
        else:
            raise ValueError(variant)
    return _cache[key]


def _mask_is_broadcast(mask2):
    # mask rows constant across HW within each channel?
    m4 = mask2.reshape(MAX_ID, C, HW)
    # sample check first to fail fast, then full check
    s = m4[::997]
    if not np.all(s == s[:, :, :1]):
        return False
    return bool(np.all(m4 == m4[:, :, :1]))


def kernel(X, idx, mask_tensor, _profile=False, _variant=None):
    assert X.shape == (B, C, H, W) and mask_tensor.shape == (MAX_ID, C, H, W)
    X2 = np.ascontiguousarray(np.asarray(X, dtype=np.float32).reshape(B, D))
    mask2 = np.asarray(mask_tensor, dtype=np.float32).reshape(MAX_ID, D)
    idx32 = np.asarray(idx).astype(np.int32).reshape(B)

    variant = _variant or os.environ.get("BASS_VARIANT")
    if variant is None:
        variant = "compact" if _mask_is_broadcast(mask2) else "fused"
    if variant == "compact":
        mask_in = np.ascontiguousarray(mask2[:, ::HW])  # [MAX_ID, C]
    else:
        mask_in = np.ascontiguousarray(mask2)

    nc = _get_nc(variant)

    in_maps = []
    for c in range(N_CORES):
        shard = slice(c * BS, (c + 1) * BS)
        idx_shard = np.ascontiguousarray(idx32[shard].reshape(NBLK, P).T)
        in_maps.append({"x": X2[shard], "idx": idx_shard, "mask": mask_in})

    res = run_bass_kernel_spmd(
        nc, in_maps, core_ids=list(range(N_CORES)), trace=_profile
    )
    out = np.concatenate([r["out"] for r in res.results], axis=0)
    if _profile:
        kernel.last_exec_time_ns = res.exec_time_ns
        kernel.last_results = res
    return out.reshape(B, C, H, W)
